# revision 13
# baseline (speedup 1.0000x reference)
"""BiMultiHeadAttention TRN2 kernel.

Sharding: 8 cores = batch (2) x tv-slice (4 x 2048 rows). Vision path fully
local; language path reduced via one AllReduce of per-head numerators over
each 4-core group. All matmuls run in float32r (full PE rate, ~1e-4 rounding).
"""

import os
import numpy as np
import concourse.bacc as bacc
import concourse.bass as bass
import concourse.tile as tile
from concourse import mybir
from concourse.bass_utils import run_bass_kernel_spmd
from concourse.masks import make_identity

f32 = mybir.dt.float32
f32r = mybir.dt.float32r
AF = mybir.ActivationFunctionType

B, TV, TL, VD, LD, E, H = 2, 8192, 256, 256, 768, 2048, 8
D = E // H  # 256
SCALE = D ** -0.5
N_CORES = 8
TVC = TV * B // N_CORES  # 2048 rows per core
TT = 512                 # t-block size
NBLK = TVC // TT         # 4 blocks
NTT = TT // 128          # 4 t-tiles per block
P = 128
EC = E // P              # 16
LC = LD // P             # 6

_CACHED = {}


def build_nc():
    nc = bacc.Bacc("TRN2", target_bir_lowering=False, debug=False,
                   num_devices=N_CORES)

    din = lambda n, s: nc.dram_tensor(n, s, f32, kind="ExternalInput").ap()
    dout = lambda n, s: nc.dram_tensor(n, s, f32, kind="ExternalOutput").ap()

    v_ap = din("v", [TVC, VD])
    l_ap = din("l", [TL, LD])
    mask_ap = din("mask", [TL, 1])
    Wv_ap = din("Wv", [VD, E])
    Wl_ap = din("Wl", [LD, E])
    Wvl_ap = din("Wvl", [LD, E])
    Wov_ap = din("Wov", [E, VD])
    Wvv_ap = din("Wvv", [VD, E])
    Wol_ap = din("Wol", [E, LD])
    bv_ap = din("bv", [E, 1])
    bl_ap = din("bl", [E, 1])
    bvl_ap = din("bvl", [E, 1])
    bvv_ap = din("bvv", [E, 1])
    bov_ap = din("bov", [1, VD])
    bol_ap = din("bol", [1, LD])

    ov_ap = dout("out_v", [TVC, VD])
    ol_ap = dout("out_l", [TL, LD])

    DBG = bool(os.environ.get("KDBG"))
    if DBG:
        dbg = {n: dout(n, s) for n, s in [
            ("dbg_q", [E, TT]), ("dbg_kT", [E, TL]), ("dbg_e1", [TL, H * TT]),
            ("dbg_e2", [TT, H * TL]), ("dbg_en", [TL, H * TT]),
            ("dbg_pm", [TL, H * VD]), ("dbg_aacc", [2 * P, H * TL]),
            ("dbg_cs", [TL, H]), ("dbg_rb", [P, TT]), ("dbg_ovt", [VD, TT]),
            ("dbg_vt", [VD, TT])]}

    with tile.TileContext(nc) as tc:
        with tc.tile_pool(name="res", bufs=1) as res, \
             tc.tile_pool(name="dram", bufs=1, space="DRAM") as dram:
            # ---------- constants ----------
            identf = res.tile([P, P], f32)
            make_identity(nc, identf)
            ones1f = res.tile([1, P], f32)
            nc.any.memset(ones1f[:], 1.0)
            ones1 = res.tile([1, P], f32r)
            nc.vector.tensor_copy(ones1[:], ones1f[:])
            onestf = res.tile([1, TT], f32)
            nc.any.memset(onestf[:], 1.0)
            onest = res.tile([1, TT], f32r)
            nc.vector.tensor_copy(onest[:], onestf[:])

            def load_cols(ap, nch, name):
                t = res.tile([P, nch], f32, tag=name, name=name)
                for j in range(nch):
                    nc.sync.dma_start(t[:, j:j + 1], ap[j * P:(j + 1) * P, :])
                return t
            bv_t = load_cols(bv_ap, EC, "bv")
            bv_s = res.tile([P, EC], f32)
            nc.vector.tensor_scalar_mul(bv_s[:], bv_t[:], SCALE)
            bl_t = load_cols(bl_ap, EC, "bl")
            bvl_t = load_cols(bvl_ap, EC, "bvl")
            bvv_t = load_cols(bvv_ap, EC, "bvv")
            bvv_r = res.tile([P, EC], f32r)
            nc.vector.tensor_copy(bvv_r[:], bvv_t[:])
            bov_f = res.tile([1, VD], f32)
            nc.sync.dma_start(bov_f[:], bov_ap[:])
            bov_r = res.tile([1, VD], f32r)
            nc.vector.tensor_copy(bov_r[:], bov_f[:])
            bol_f = res.tile([1, LD], f32)
            nc.sync.dma_start(bol_f[:], bol_ap[:])

            maskf = res.tile([P, 2], f32)
            for sc in range(2):
                nc.sync.dma_start(maskf[:, sc:sc + 1],
                                  mask_ap[sc * P:(sc + 1) * P, :])
            maskr = res.tile([P, 2], f32r)
            nc.vector.tensor_copy(maskr[:], maskf[:])

            # ---------- resident tensors ----------
            mid = tc.tile_pool(name="mid", bufs=1)
            midp = mid.__enter__()
            Wv_r = [midp.tile([P, E], f32r, tag=f"wv{k}", name=f"wv{k}")
                    for k in range(2)]
            kT = [midp.tile([P, TL], f32r, tag=f"kT{i}", name=f"kT{i}")
                  for i in range(EC)]
            Pm = [midp.tile([P, H * VD], f32r, tag=f"pm{s}", name=f"pm{s}")
                  for s in range(2)]
            Aacc = [res.tile([P, H * TL], f32, tag=f"aacc{d}", name=f"aacc{d}")
                    for d in range(2)]
            csp = [res.tile([P, H * NBLK], f32, tag=f"csp{s}", name=f"csp{s}")
                   for s in range(2)]
            cs2 = [res.tile([P, H], f32, tag=f"cs2{s}", name=f"cs2{s}")
                   for s in range(2)]

            ccw = H * TL + H
            cc_in = dram.tile([2, P, ccw], f32)
            cc_out = dram.tile([2, P, ccw], f32)

            # ---------- phase 0: language-side precompute ----------
            with tc.tile_pool(name="ph0", bufs=2) as ph0, \
                 tc.tile_pool(name="ph0a", bufs=1) as ph0a, \
                 tc.tile_pool(name="ph0w", bufs=LC) as ph0w, \
                 tc.tile_pool(name="ps0", bufs=3, space="PSUM") as ps0:
                for k in range(2):
                    st = ph0.tile([P, E], f32, tag="wvstage")
                    nc.sync.dma_start(st[:], Wv_ap[k * P:(k + 1) * P, :])
                    nc.vector.tensor_copy(Wv_r[k][:], st[:])

                lT = [ph0a.tile([P, TL], f32r, tag=f"lT{c}", name=f"lT{c}")
                      for c in range(LC)]
                for sc in range(2):
                    lch = ph0.tile([P, LD], f32, tag="lstage")
                    nc.sync.dma_start(lch[:], l_ap[sc * P:(sc + 1) * P, :])
                    for cb in range(LC):
                        pt = ps0.tile([P, P], f32, tag="mm")
                        nc.tensor.transpose(pt[:], lch[:, cb * P:(cb + 1) * P],
                                            identf[:])
                        nc.any.tensor_copy(lT[cb][:, sc * P:(sc + 1) * P], pt[:])

                vlT = [ph0a.tile([P, TL], f32r, tag=f"vlT{c}", name=f"vlT{c}")
                       for c in range(EC)]
                for (W_ap, bias_t, dst) in ((Wl_ap, bl_t, kT), (Wvl_ap, bvl_t, vlT)):
                    wst_r = []
                    for c in range(LC):
                        st = ph0.tile([P, E], f32, tag="wstage")
                        nc.sync.dma_start(st[:], W_ap[c * P:(c + 1) * P, :])
                        rt = ph0w.tile([P, E], f32r, tag="wstr")
                        nc.vector.tensor_copy(rt[:], st[:])
                        wst_r.append(rt)
                    for m in range(EC):
                        pk = ps0.tile([P, TL], f32, tag="mm")
                        for c in range(LC):
                            nc.tensor.matmul(pk[:], wst_r[c][:, m * P:(m + 1) * P],
                                             lT[c][:], start=(c == 0),
                                             stop=(c == LC - 1))
                        nc.scalar.activation(dst[m][:], pk[:], AF.Identity,
                                             bias=bias_t[:, m:m + 1])

                for h in range(H):
                    wov_r = []
                    for d in range(2):
                        st = ph0.tile([P, VD], f32, tag="wovst")
                        nc.sync.dma_start(
                            st[:], Wov_ap[h * D + d * P:h * D + (d + 1) * P, :])
                        rt = ph0.tile([P, VD], f32r, tag="wovr")
                        nc.vector.tensor_copy(rt[:], st[:])
                        wov_r.append(rt)
                    for sc in range(2):
                        pp = ps0.tile([P, VD], f32, tag="mm")
                        for d in range(2):
                            nc.tensor.matmul(
                                pp[:], vlT[2 * h + d][:, sc * P:(sc + 1) * P],
                                wov_r[d][:], start=(d == 0), stop=(d == 1))
                        nc.vector.tensor_scalar_mul(
                            Pm[sc][:, h * VD:(h + 1) * VD], pp[:],
                            maskf[:, sc:sc + 1])

            if DBG:
                for i in range(EC):
                    nc.sync.dma_start(dbg["dbg_kT"][i * P:(i + 1) * P, :],
                                      kT[i].bitcast(f32)[:])
                for sc in range(2):
                    nc.sync.dma_start(dbg["dbg_pm"][sc * P:(sc + 1) * P, :],
                                      Pm[sc].bitcast(f32)[:])

            # ---------- main block loop over t ----------
            with tc.tile_pool(name="blk", bufs=2) as bp, \
                 tc.tile_pool(name="blk1", bufs=1) as bp1, \
                 tc.tile_pool(name="psA", bufs=2, space="PSUM") as psA, \
                 tc.tile_pool(name="ps1", bufs=3, space="PSUM") as ps1, \
                 tc.tile_pool(name="psr", bufs=1, space="PSUM") as psr:
                for blk in range(NBLK):
                    t0 = blk * TT
                    v_f = [bp.tile([P, VD], f32, tag=f"vf{tt}", name=f"vf{tt}")
                           for tt in range(NTT)]
                    v_r = [bp1.tile([P, VD], f32r, tag=f"vr{tt}", name=f"vr{tt}")
                           for tt in range(NTT)]
                    vT = [bp1.tile([P, TT], f32r, tag=f"vT{d}", name=f"vT{d}")
                          for d in range(2)]
                    for tt in range(NTT):
                        nc.sync.dma_start(v_f[tt][:],
                                          v_ap[t0 + tt * P:t0 + (tt + 1) * P, :])
                        nc.vector.tensor_copy(v_r[tt][:], v_f[tt][:])
                        for d in range(2):
                            pt = ps1.tile([P, P], f32, tag="mm")
                            nc.tensor.transpose(
                                pt[:], v_f[tt][:, d * P:(d + 1) * P], identf[:])
                            nc.any.tensor_copy(vT[d][:, tt * P:(tt + 1) * P], pt[:])

                    qT = [bp1.tile([P, TT], f32r, tag=f"qT{m}", name=f"qT{m}")
                          for m in range(EC)]
                    for m in range(EC):
                        pq = ps1.tile([P, TT], f32, tag="mm")
                        for k in range(2):
                            nc.tensor.matmul(pq[:], Wv_r[k][:, m * P:(m + 1) * P],
                                             vT[k][:], start=(k == 0), stop=(k == 1))
                        nc.scalar.activation(qT[m][:], pq[:], AF.Identity,
                                             bias=bv_s[:, m:m + 1], scale=SCALE)
                    if DBG and blk == 0:
                        for m in range(EC):
                            nc.sync.dma_start(dbg["dbg_q"][m * P:(m + 1) * P, :],
                                              qT[m].bitcast(f32)[:])
                        for dd in range(2):
                            nc.sync.dma_start(dbg["dbg_vt"][dd * P:(dd + 1) * P, :],
                                              vT[dd].bitcast(f32)[:])

                    exp1 = [bp1.tile([P, H * TT], f32r, tag=f"e1{sc}",
                                     name=f"e1{sc}") for sc in range(2)]
                    for sc in range(2):
                        for h in range(H):
                            pa = ps1.tile([P, TT], f32, tag="mm")
                            for k in range(2):
                                nc.tensor.matmul(
                                    pa[:],
                                    kT[2 * h + k][:, sc * P:(sc + 1) * P],
                                    qT[2 * h + k][:], start=(k == 0), stop=(k == 1))
                            nc.scalar.activation(
                                exp1[sc][:, h * TT:(h + 1) * TT], pa[:], AF.Exp,
                                accum_out=csp[sc][:, h * NBLK + blk:
                                                  h * NBLK + blk + 1])
                    if DBG and blk == 0:
                        for sc in range(2):
                            nc.sync.dma_start(dbg["dbg_e1"][sc * P:(sc + 1) * P, :],
                                              exp1[sc].bitcast(f32)[:])

                    exp2 = [bp.tile([P, H * TL], f32r, tag=f"e2{tt % 2}",
                                    name=f"e2{tt}") for tt in range(NTT)]
                    for tt in range(NTT):
                        for h in range(H):
                            pa = ps1.tile([P, TL], f32, tag="mm")
                            for k in range(2):
                                nc.tensor.matmul(
                                    pa[:],
                                    qT[2 * h + k][:, tt * P:(tt + 1) * P],
                                    kT[2 * h + k][:], start=(k == 0), stop=(k == 1))
                            nc.scalar.activation(
                                exp2[tt][:, h * TL:(h + 1) * TL], pa[:], AF.Exp)
                    if DBG and blk == 0:
                        for tt in range(NTT):
                            nc.sync.dma_start(
                                dbg["dbg_e2"][tt * P:(tt + 1) * P, :],
                                exp2[tt].bitcast(f32)[:])

                    # A^T accumulation: head-pair packed (N=512, start per bank)
                    for qd in range(2):
                        pA = [psA.tile([P, 4 * TL], f32, tag="pA", name="pA")
                              for d in range(2)]
                        for d in range(2):
                            for tt in range(NTT):
                                for hp in range(2):
                                    h0 = qd * 4 + hp * 2
                                    nc.tensor.matmul(
                                        pA[d][:, hp * 2 * TL:(hp + 1) * 2 * TL],
                                        v_r[tt][:, d * P:(d + 1) * P],
                                        exp2[tt][:, h0 * TL:(h0 + 2) * TL],
                                        start=(tt == 0),
                                        stop=(tt == NTT - 1),
                                        skip_group_check=True)
                            dst = Aacc[d][:, qd * 4 * TL:(qd + 1) * 4 * TL]
                            if blk == 0:
                                nc.vector.tensor_copy(dst, pA[d][:])
                            else:
                                nc.vector.tensor_add(dst, pA[d][:], dst)

                    if blk == NBLK - 1:
                        for sc in range(2):
                            nc.vector.tensor_reduce(
                                cs2[sc][:],
                                csp[sc][:].rearrange("p (h b) -> p h b", h=H),
                                axis=mybir.AxisListType.X, op=mybir.AluOpType.add)
                        for d in range(2):
                            for qq in range(4):
                                eng = (nc.gpsimd, nc.scalar, nc.gpsimd,
                                       nc.scalar)[qq]
                                sl = slice(qq * 2 * TL, (qq + 1) * 2 * TL)
                                eng.dma_start(cc_in[d, :, sl], Aacc[d][:, sl])
                        for sc in range(2):
                            nc.gpsimd.dma_start(cc_in[sc, :, H * TL:ccw], cs2[sc][:])
                        nc.gpsimd.collective_compute(
                            "AllReduce", mybir.AluOpType.add,
                            replica_groups=[[0, 1, 2, 3], [4, 5, 6, 7]],
                            ins=[cc_in.opt()], outs=[cc_out.opt()])

                    # masked row-sums -> recip broadcast -> normalize exp1 in place
                    for h in range(H):
                        prs = psr.tile([1, TT], f32, tag="rs")
                        for sc in range(2):
                            nc.tensor.matmul(prs[:], maskr[:, sc:sc + 1],
                                             exp1[sc][:, h * TT:(h + 1) * TT],
                                             start=(sc == 0), stop=(sc == 1))
                        rs_row = bp.tile([1, TT], f32, tag="rsrow")
                        nc.scalar.copy(rs_row[:], prs[:])
                        rbB = bp.tile([P, TT], f32, tag="rbB")
                        nc.gpsimd.partition_broadcast(rbB[:], rs_row[:])
                        rb = bp.tile([P, TT], f32, tag="rb")
                        nc.vector.reciprocal_approx_fast(rb[:], rbB[:])
                        if DBG and blk == 0 and h == 0:
                            nc.sync.dma_start(dbg["dbg_rb"][:], rb[:])
                        for sc in range(2):
                            sl = exp1[sc][:, h * TT:(h + 1) * TT]
                            nc.vector.tensor_mul(sl, sl, rb[:])
                    if DBG and blk == 0:
                        for sc in range(2):
                            nc.sync.dma_start(dbg["dbg_en"][sc * P:(sc + 1) * P, :],
                                              exp1[sc].bitcast(f32)[:])

                    # out_v^T accumulate all heads + bov, then transpose out
                    ovT = [bp1.tile([P, TT], f32, tag=f"ovT{mv}", name=f"ovT{mv}")
                           for mv in range(2)]
                    for mv in range(2):
                        po = ps1.tile([P, TT], f32, tag="mm")
                        for h in range(H):
                            for sc in range(2):
                                nc.tensor.matmul(
                                    po[:],
                                    Pm[sc][:, h * VD + mv * P:h * VD + (mv + 1) * P],
                                    exp1[sc][:, h * TT:(h + 1) * TT],
                                    start=(h == 0 and sc == 0), stop=False)
                        nc.tensor.matmul(po[:], bov_r[:, mv * P:(mv + 1) * P],
                                         onest[:], start=False, stop=True)
                        nc.any.tensor_copy(ovT[mv][:], po[:])
                    if DBG and blk == 0:
                        for mv in range(2):
                            nc.sync.dma_start(dbg["dbg_ovt"][mv * P:(mv + 1) * P, :],
                                              ovT[mv][:])
                    for tt in range(NTT):
                        ovf = bp.tile([P, VD], f32, tag="ovf")
                        for mv in range(2):
                            pt = ps1.tile([P, P], f32, tag="mm")
                            nc.tensor.transpose(
                                pt[:], ovT[mv][:, tt * P:(tt + 1) * P], identf[:])
                            nc.any.tensor_copy(ovf[:, mv * P:(mv + 1) * P], pt[:])
                        nc.sync.dma_start(
                            ov_ap[t0 + tt * P:t0 + (tt + 1) * P, :], ovf[:])

            # ---------- collective (emitted inside last block) ----------
            if DBG:
                for d in range(2):
                    nc.sync.dma_start(dbg["dbg_aacc"][d * P:(d + 1) * P, :],
                                      Aacc[d][:])
                for sc in range(2):
                    nc.sync.dma_start(dbg["dbg_cs"][sc * P:(sc + 1) * P, :],
                                      cs2[sc][:])
            mid.__exit__(None, None, None)

            # ---------- end phase: out_l ----------
            with tc.tile_pool(name="endp", bufs=2) as ep, \
                 tc.tile_pool(name="endp1", bufs=1) as ep1, \
                 tc.tile_pool(name="pse", bufs=2, space="PSUM") as pse, \
                 tc.tile_pool(name="pser", bufs=2, space="PSUM") as pser:
                Ag = [ep1.tile([P, H * TL], f32, tag=f"ag{d}", name=f"ag{d}")
                      for d in range(2)]
                for d in range(2):
                    for qq in range(4):
                        eng = (nc.gpsimd, nc.scalar, nc.gpsimd, nc.sync)[qq]
                        sl = slice(qq * 2 * TL, (qq + 1) * 2 * TL)
                        eng.dma_start(Ag[d][:, sl], cc_out[d, :, sl])
                csg = [ep1.tile([P, H], f32, tag=f"csg{s}", name=f"csg{s}")
                       for s in range(2)]
                for sc in range(2):
                    nc.sync.dma_start(csg[sc][:], cc_out[sc, :, H * TL:ccw])

                csT = [ep1.tile([H, P], f32, tag=f"csT{s}", name=f"csT{s}")
                       for s in range(2)]
                for sc in range(2):
                    pt = pse.tile([H, P], f32, tag="mm")
                    nc.tensor.transpose(pt[:], csg[sc][:], identf[:])
                    nc.any.tensor_copy(csT[sc][:], pt[:])
                csrow = ep1.tile([H, TL], f32, tag="csrow")
                for sc in range(2):
                    nc.sync.dma_start(csrow[:, sc * P:(sc + 1) * P], csT[sc][:])
                crec = ep1.tile([H, TL], f32, tag="crec")
                nc.vector.reciprocal_approx_fast(crec[:], csrow[:])

                Ab = [ep1.tile([P, H * TL], f32r, tag=f"ab{d}", name=f"ab{d}")
                      for d in range(2)]
                for h in range(H):
                    crowf = ep.tile([1, TL], f32, tag="crowf", name="crowf")
                    nc.sync.dma_start(crowf[:], crec[h:h + 1, :])
                    pb = ep.tile([P, TL], f32, tag="pbb", name="pbb")
                    nc.gpsimd.partition_broadcast(pb[:], crowf[:])
                    for d in range(2):
                        nc.vector.tensor_mul(Ab[d][:, h * TL:(h + 1) * TL],
                                             Ag[d][:, h * TL:(h + 1) * TL], pb[:])

                Wvv_r = []
                for d in range(2):
                    st = ep.tile([P, E], f32, tag="wvvst")
                    nc.sync.dma_start(st[:], Wvv_ap[d * P:(d + 1) * P, :])
                    rt = ep1.tile([P, E], f32r, tag=f"wvvr{d}", name=f"wvvr{d}")
                    nc.vector.tensor_copy(rt[:], st[:])
                    Wvv_r.append(rt)
                vvT = [ep1.tile([P, TL], f32r, tag=f"vvT{m}", name=f"vvT{m}")
                       for m in range(EC)]
                for h in range(H):
                    for me in range(2):
                        pv = pse.tile([P, TL], f32, tag="mm")
                        for d in range(2):
                            nc.tensor.matmul(
                                pv[:],
                                Wvv_r[d][:, h * D + me * P:h * D + (me + 1) * P],
                                Ab[d][:, h * TL:(h + 1) * TL],
                                start=(d == 0), stop=(d == 1))
                        nc.any.tensor_copy(vvT[2 * h + me][:], pv[:])

                NB = 2
                NW = LD // NB  # 384
                pc = [pser.tile([1, NW], f32, tag="rs", name="pcrow")
                      for _ in range(NB)]
                po = [pse.tile([P, NW], f32, tag="po", name="po", bufs=4)
                      for _ in range(4)]
                for c in range(EC):
                    st = ep.tile([P, LD], f32, tag="wolst", name="wolst", bufs=3)
                    nc.sync.dma_start(st[:], Wol_ap[c * P:(c + 1) * P, :])
                    rt = ep.tile([P, LD], f32r, tag="wolr", name="wolr", bufs=3)
                    nc.vector.tensor_copy(rt[:], st[:])
                    for nb in range(NB):
                        nc.tensor.matmul(pc[nb][:], bvv_r[:, c:c + 1],
                                         rt[:, nb * NW:(nb + 1) * NW],
                                         start=(c == 0), stop=(c == EC - 1))
                        for sh in range(2):
                            nc.tensor.matmul(
                                po[sh * NB + nb][:], vvT[c][:, sh * P:(sh + 1) * P],
                                rt[:, nb * NW:(nb + 1) * NW],
                                start=(c == 0), stop=False)
                c_f = ep1.tile([1, LD], f32, tag="cf")
                for nb in range(NB):
                    nc.scalar.copy(c_f[:, nb * NW:(nb + 1) * NW], pc[nb][:])
                nc.vector.tensor_add(c_f[:], c_f[:], bol_f[:])
                c_r = ep1.tile([1, LD], f32r, tag="cr")
                nc.vector.tensor_copy(c_r[:], c_f[:])

                for sh in range(2):
                    olf = ep1.tile([P, LD], f32, tag=f"olf{sh}", name=f"olf{sh}")
                    for nb in range(NB):
                        nc.tensor.matmul(po[sh * NB + nb][:], ones1[:],
                                         c_r[:, nb * NW:(nb + 1) * NW],
                                         start=False, stop=True)
                        nc.any.tensor_copy(olf[:, nb * NW:(nb + 1) * NW],
                                           po[sh * NB + nb][:])
                    nc.sync.dma_start(ol_ap[sh * P:(sh + 1) * P, :], olf[:])

    nc.compile()
    return nc


def kernel(v, l, attention_mask_l, Wv, bv, Wl, bl, Wvv, bvv, Wvl, bvl,
           Wov, bov, Wol, bol):
    if "nc" not in _CACHED:
        _CACHED["nc"] = build_nc()
    nc = _CACHED["nc"]

    v = np.asarray(v, dtype=np.float32)
    l = np.asarray(l, dtype=np.float32)
    maskf = np.asarray(attention_mask_l).astype(np.float32)
    w = {k: np.ascontiguousarray(np.asarray(x, dtype=np.float32))
         for k, x in (("Wv", Wv), ("Wl", Wl), ("Wvl", Wvl), ("Wov", Wov),
                      ("Wvv", Wvv), ("Wol", Wol))}
    bias = {"bv": bv, "bl": bl, "bvl": bvl, "bvv": bvv}
    bias = {k: np.ascontiguousarray(np.asarray(x, np.float32).reshape(E, 1))
            for k, x in bias.items()}
    bov_m = np.ascontiguousarray(np.asarray(bov, np.float32).reshape(1, VD))
    bol_m = np.ascontiguousarray(np.asarray(bol, np.float32).reshape(1, LD))

    in_maps = []
    for c in range(N_CORES):
        b = c // 4
        s = c % 4
        m = {
            "v": np.ascontiguousarray(v[b, s * TVC:(s + 1) * TVC, :]),
            "l": np.ascontiguousarray(l[b]),
            "mask": np.ascontiguousarray(maskf[b].reshape(TL, 1)),
            "bov": bov_m, "bol": bol_m,
        }
        m.update(w)
        m.update(bias)
        in_maps.append(m)

    _CACHED["in_maps"] = in_maps
    res = run_bass_kernel_spmd(nc, in_maps, core_ids=list(range(N_CORES)))
    out_v = np.empty((B, TV, VD), np.float32)
    for c in range(N_CORES):
        b, s = c // 4, c % 4
        out_v[b, s * TVC:(s + 1) * TVC, :] = res.results[c]["out_v"]
    out_l = np.stack([res.results[0]["out_l"], res.results[4]["out_l"]])
    return out_v, out_l


# revision 15
# speedup vs baseline: 1.0691x; 1.0691x over previous
"""BiMultiHeadAttention TRN2 kernel.

Sharding: 8 cores = batch (2) x tv-slice (4 x 2048 rows). Vision path fully
local; language path reduced via one AllReduce of per-head numerators over
each 4-core group. All matmuls run in float32r (full PE rate, ~1e-4 rounding).
"""

import os
import numpy as np
import concourse.bacc as bacc
import concourse.bass as bass
import concourse.tile as tile
from concourse import mybir
from concourse.bass_utils import run_bass_kernel_spmd
from concourse.masks import make_identity

f32 = mybir.dt.float32
f32r = mybir.dt.float32r
AF = mybir.ActivationFunctionType

B, TV, TL, VD, LD, E, H = 2, 8192, 256, 256, 768, 2048, 8
D = E // H  # 256
SCALE = D ** -0.5
N_CORES = 8
TVC = TV * B // N_CORES  # 2048 rows per core
TT = 512                 # t-block size
NBLK = TVC // TT         # 4 blocks
NTT = TT // 128          # 4 t-tiles per block
P = 128
EC = E // P              # 16
LC = LD // P             # 6

_CACHED = {}


def build_nc():
    nc = bacc.Bacc("TRN2", target_bir_lowering=False, debug=False,
                   num_devices=N_CORES)

    din = lambda n, s: nc.dram_tensor(n, s, f32, kind="ExternalInput").ap()
    dout = lambda n, s: nc.dram_tensor(n, s, f32, kind="ExternalOutput").ap()

    v_ap = din("v", [TVC, VD])
    l_ap = din("l", [TL, LD])
    mask_ap = din("mask", [TL, 1])
    Wv_ap = din("Wv", [VD, E])
    Wl_ap = din("Wl", [LD, E])
    Wvl_ap = din("Wvl", [LD, E])
    Wov_ap = din("Wov", [E, VD])
    Wvv_ap = din("Wvv", [VD, E])
    Wol_ap = din("Wol", [E, LD])
    bv_ap = din("bv", [E, 1])
    bl_ap = din("bl", [E, 1])
    bvl_ap = din("bvl", [E, 1])
    bvv_ap = din("bvv", [E, 1])
    bov_ap = din("bov", [1, VD])
    bol_ap = din("bol", [1, LD])

    ov_ap = dout("out_v", [TVC, VD])
    ol_ap = dout("out_l", [TL, LD])

    DBG = bool(os.environ.get("KDBG"))
    if DBG:
        dbg = {n: dout(n, s) for n, s in [
            ("dbg_q", [E, TT]), ("dbg_kT", [E, TL]), ("dbg_e1", [TL, H * TT]),
            ("dbg_e2", [TT, H * TL]), ("dbg_en", [TL, H * TT]),
            ("dbg_pm", [TL, H * VD]), ("dbg_aacc", [2 * P, H * TL]),
            ("dbg_cs", [TL, H]), ("dbg_rb", [P, TT]), ("dbg_ovt", [VD, TT]),
            ("dbg_vt", [VD, TT])]}

    with tile.TileContext(nc) as tc:
        with tc.tile_pool(name="res", bufs=1) as res, \
             tc.tile_pool(name="dram", bufs=1, space="DRAM") as dram:
            # ---------- constants ----------
            identf = res.tile([P, P], f32)
            make_identity(nc, identf)
            ones1f = res.tile([1, P], f32)
            nc.any.memset(ones1f[:], 1.0)
            ones1 = res.tile([1, P], f32r)
            nc.vector.tensor_copy(ones1[:], ones1f[:])
            onestf = res.tile([1, TT], f32)
            nc.any.memset(onestf[:], 1.0)
            onest = res.tile([1, TT], f32r)
            nc.vector.tensor_copy(onest[:], onestf[:])

            def load_cols(ap, nch, name):
                t = res.tile([P, nch], f32, tag=name, name=name)
                for j in range(nch):
                    nc.scalar.dma_start(t[:, j:j + 1], ap[j * P:(j + 1) * P, :])
                return t
            bv_t = load_cols(bv_ap, EC, "bv")
            bv_s = res.tile([P, EC], f32)
            nc.vector.tensor_scalar_mul(bv_s[:], bv_t[:], SCALE)
            bl_t = load_cols(bl_ap, EC, "bl")
            bvl_t = load_cols(bvl_ap, EC, "bvl")
            bvv_t = load_cols(bvv_ap, EC, "bvv")
            bvv_r = res.tile([P, EC], f32r)
            nc.vector.tensor_copy(bvv_r[:], bvv_t[:])
            bov_f = res.tile([1, VD], f32)
            nc.scalar.dma_start(bov_f[:], bov_ap[:])
            bov_r = res.tile([1, VD], f32r)
            nc.vector.tensor_copy(bov_r[:], bov_f[:])
            bol_f = res.tile([1, LD], f32)
            nc.scalar.dma_start(bol_f[:], bol_ap[:])

            maskf = res.tile([P, 2], f32)
            for sc in range(2):
                nc.scalar.dma_start(maskf[:, sc:sc + 1],
                                    mask_ap[sc * P:(sc + 1) * P, :])
            maskr = res.tile([P, 2], f32r)
            nc.vector.tensor_copy(maskr[:], maskf[:])

            # ---------- resident tensors ----------
            mid = tc.tile_pool(name="mid", bufs=1)
            midp = mid.__enter__()
            Wv_r = [midp.tile([P, E], f32r, tag=f"wv{k}", name=f"wv{k}")
                    for k in range(2)]
            kT = [midp.tile([P, TL], f32r, tag=f"kT{i}", name=f"kT{i}")
                  for i in range(EC)]
            Pm = [midp.tile([P, H * VD], f32r, tag=f"pm{s}", name=f"pm{s}")
                  for s in range(2)]
            Aacc = [res.tile([P, H * TL], f32, tag=f"aacc{d}", name=f"aacc{d}")
                    for d in range(2)]
            csp = [res.tile([P, H * NBLK], f32, tag=f"csp{s}", name=f"csp{s}")
                   for s in range(2)]
            cs2 = [res.tile([P, H], f32, tag=f"cs2{s}", name=f"cs2{s}")
                   for s in range(2)]

            ccw = H * TL + H
            cc_in = dram.tile([2, P, ccw], f32)
            cc_out = dram.tile([2, P, ccw], f32)

            # ---------- phase 0: language-side precompute ----------
            with tc.tile_pool(name="ph0", bufs=2) as ph0, \
                 tc.tile_pool(name="ph0a", bufs=1) as ph0a, \
                 tc.tile_pool(name="ph0w", bufs=LC) as ph0w, \
                 tc.tile_pool(name="ps0", bufs=3, space="PSUM") as ps0:
                for k in range(2):
                    st = ph0.tile([P, E], f32, tag="wvstage")
                    nc.sync.dma_start(st[:], Wv_ap[k * P:(k + 1) * P, :])
                    nc.vector.tensor_copy(Wv_r[k][:], st[:])

                lT = [ph0a.tile([P, TL], f32r, tag=f"lT{c}", name=f"lT{c}")
                      for c in range(LC)]
                for sc in range(2):
                    lch = ph0.tile([P, LD], f32, tag="lstage")
                    nc.sync.dma_start(lch[:], l_ap[sc * P:(sc + 1) * P, :])
                    for cb in range(LC):
                        pt = ps0.tile([P, P], f32, tag="mm")
                        nc.tensor.transpose(pt[:], lch[:, cb * P:(cb + 1) * P],
                                            identf[:])
                        nc.any.tensor_copy(lT[cb][:, sc * P:(sc + 1) * P], pt[:])

                vlT = [ph0a.tile([P, TL], f32r, tag=f"vlT{c}", name=f"vlT{c}")
                       for c in range(EC)]
                for (W_ap, bias_t, dst) in ((Wl_ap, bl_t, kT), (Wvl_ap, bvl_t, vlT)):
                    wst_r = []
                    for c in range(LC):
                        st = ph0.tile([P, E], f32, tag="wstage")
                        nc.sync.dma_start(st[:], W_ap[c * P:(c + 1) * P, :])
                        rt = ph0w.tile([P, E], f32r, tag="wstr")
                        nc.vector.tensor_copy(rt[:], st[:])
                        wst_r.append(rt)
                    for m in range(EC):
                        pk = ps0.tile([P, TL], f32, tag="mm")
                        for c in range(LC):
                            nc.tensor.matmul(pk[:], wst_r[c][:, m * P:(m + 1) * P],
                                             lT[c][:], start=(c == 0),
                                             stop=(c == LC - 1))
                        nc.vector.tensor_scalar_add(dst[m][:], pk[:],
                                                    bias_t[:, m:m + 1])

                for h in range(H):
                    wov_r = []
                    for d in range(2):
                        st = ph0.tile([P, VD], f32, tag="wovst")
                        nc.sync.dma_start(
                            st[:], Wov_ap[h * D + d * P:h * D + (d + 1) * P, :])
                        rt = ph0.tile([P, VD], f32r, tag="wovr")
                        nc.vector.tensor_copy(rt[:], st[:])
                        wov_r.append(rt)
                    for sc in range(2):
                        pp = ps0.tile([P, VD], f32, tag="mm")
                        for d in range(2):
                            nc.tensor.matmul(
                                pp[:], vlT[2 * h + d][:, sc * P:(sc + 1) * P],
                                wov_r[d][:], start=(d == 0), stop=(d == 1))
                        nc.vector.tensor_scalar_mul(
                            Pm[sc][:, h * VD:(h + 1) * VD], pp[:],
                            maskf[:, sc:sc + 1])

            if DBG:
                for i in range(EC):
                    nc.sync.dma_start(dbg["dbg_kT"][i * P:(i + 1) * P, :],
                                      kT[i].bitcast(f32)[:])
                for sc in range(2):
                    nc.sync.dma_start(dbg["dbg_pm"][sc * P:(sc + 1) * P, :],
                                      Pm[sc].bitcast(f32)[:])

            # ---------- main block loop over t ----------
            with tc.tile_pool(name="blk", bufs=2) as bp, \
                 tc.tile_pool(name="blk1", bufs=1) as bp1, \
                 tc.tile_pool(name="psA", bufs=2, space="PSUM") as psA, \
                 tc.tile_pool(name="ps1", bufs=2, space="PSUM") as ps1:
                for blk in range(NBLK):
                    t0 = blk * TT
                    v_f = [bp.tile([P, VD], f32, tag=f"vf{tt}", name=f"vf{tt}")
                           for tt in range(NTT)]
                    v_r = [bp1.tile([P, VD], f32r, tag=f"vr{tt}", name=f"vr{tt}")
                           for tt in range(NTT)]
                    vT = [bp1.tile([P, TT], f32r, tag=f"vT{d}", name=f"vT{d}")
                          for d in range(2)]
                    for tt in range(NTT):
                        nc.sync.dma_start(v_f[tt][:],
                                          v_ap[t0 + tt * P:t0 + (tt + 1) * P, :])
                        nc.vector.tensor_copy(v_r[tt][:], v_f[tt][:])
                        for d in range(2):
                            pt = ps1.tile([P, P], f32, tag="mm")
                            nc.tensor.transpose(
                                pt[:], v_f[tt][:, d * P:(d + 1) * P], identf[:])
                            nc.any.tensor_copy(vT[d][:, tt * P:(tt + 1) * P], pt[:])

                    qT = [bp1.tile([P, TT], f32r, tag=f"qT{m}", name=f"qT{m}")
                          for m in range(EC)]
                    for m in range(EC):
                        pq = ps1.tile([P, TT], f32, tag="mm")
                        for k in range(2):
                            nc.tensor.matmul(pq[:], Wv_r[k][:, m * P:(m + 1) * P],
                                             vT[k][:], start=(k == 0), stop=(k == 1))
                        nc.vector.tensor_scalar(
                            qT[m][:], pq[:], bv_t[:, m:m + 1], SCALE,
                            op0=mybir.AluOpType.add, op1=mybir.AluOpType.mult)
                    if DBG and blk == 0:
                        for m in range(EC):
                            nc.sync.dma_start(dbg["dbg_q"][m * P:(m + 1) * P, :],
                                              qT[m].bitcast(f32)[:])
                        for dd in range(2):
                            nc.sync.dma_start(dbg["dbg_vt"][dd * P:(dd + 1) * P, :],
                                              vT[dd].bitcast(f32)[:])

                    exp1 = [bp1.tile([P, H * TT], f32r, tag=f"e1{sc}",
                                     name=f"e1{sc}") for sc in range(2)]
                    for sc in range(2):
                        for h in range(H):
                            pa = ps1.tile([P, TT], f32, tag="mm")
                            for k in range(2):
                                nc.tensor.matmul(
                                    pa[:],
                                    kT[2 * h + k][:, sc * P:(sc + 1) * P],
                                    qT[2 * h + k][:], start=(k == 0), stop=(k == 1))
                            nc.scalar.activation(
                                exp1[sc][:, h * TT:(h + 1) * TT], pa[:], AF.Exp,
                                accum_out=csp[sc][:, h * NBLK + blk:
                                                  h * NBLK + blk + 1])
                    if DBG and blk == 0:
                        for sc in range(2):
                            nc.sync.dma_start(dbg["dbg_e1"][sc * P:(sc + 1) * P, :],
                                              exp1[sc].bitcast(f32)[:])

                    exp2 = [bp.tile([P, H * TL], f32r, tag=f"e2{tt % 2}",
                                    name=f"e2{tt}") for tt in range(NTT)]
                    for tt in range(NTT):
                        for qh in range(2):
                            pa = psA.tile([P, 4 * TL], f32, tag="a2w",
                                          name="a2w", bufs=2)
                            for hh in range(4):
                                h = qh * 4 + hh
                                for k in range(2):
                                    nc.tensor.matmul(
                                        pa[:, hh * TL:(hh + 1) * TL],
                                        qT[2 * h + k][:, tt * P:(tt + 1) * P],
                                        kT[2 * h + k][:],
                                        start=(k == 0 and hh % 2 == 0),
                                        stop=(k == 1),
                                        skip_group_check=True)
                            nc.scalar.activation(
                                exp2[tt][:, qh * 4 * TL:(qh + 1) * 4 * TL],
                                pa[:], AF.Exp)
                    if DBG and blk == 0:
                        for tt in range(NTT):
                            nc.sync.dma_start(
                                dbg["dbg_e2"][tt * P:(tt + 1) * P, :],
                                exp2[tt].bitcast(f32)[:])

                    # A^T accumulation: head-pair packed (N=512, start per bank)
                    for d in range(2):
                        for pr in range(4):
                            pA = psA.tile([P, 2 * TL], f32, tag="pA", name="pA")
                            for tt in range(NTT):
                                nc.tensor.matmul(
                                    pA[:],
                                    v_r[tt][:, d * P:(d + 1) * P],
                                    exp2[tt][:, pr * 2 * TL:(pr + 1) * 2 * TL],
                                    start=(tt == 0), stop=(tt == NTT - 1))
                            dst = Aacc[d][:, pr * 2 * TL:(pr + 1) * 2 * TL]
                            if blk == 0:
                                nc.vector.tensor_copy(dst, pA[:])
                            else:
                                nc.vector.tensor_add(dst, pA[:], dst)
                            if blk == NBLK - 1:
                                nc.gpsimd.dma_start(
                                    cc_in[d, :, pr * 2 * TL:(pr + 1) * 2 * TL],
                                    dst)

                    if blk == NBLK - 1:
                        for sc in range(2):
                            nc.vector.tensor_reduce(
                                cs2[sc][:],
                                csp[sc][:].rearrange("p (h b) -> p h b", h=H),
                                axis=mybir.AxisListType.X, op=mybir.AluOpType.add)
                        for sc in range(2):
                            nc.gpsimd.dma_start(cc_in[sc, :, H * TL:ccw], cs2[sc][:])
                        nc.gpsimd.collective_compute(
                            "AllReduce", mybir.AluOpType.add,
                            replica_groups=[[0, 1, 2, 3], [4, 5, 6, 7]],
                            ins=[cc_in.opt()], outs=[cc_out.opt()])

                    # masked row-sums -> recip broadcast -> normalize exp1 in place
                    for h in range(H):
                        prs = ps1.tile([1, TT], f32, tag="mm")
                        for sc in range(2):
                            nc.tensor.matmul(prs[:], maskr[:, sc:sc + 1],
                                             exp1[sc][:, h * TT:(h + 1) * TT],
                                             start=(sc == 0), stop=(sc == 1))
                        rs_row = bp.tile([1, TT], f32, tag="rsrow")
                        nc.scalar.copy(rs_row[:], prs[:])
                        rbB = bp.tile([P, TT], f32, tag="rbB")
                        nc.gpsimd.partition_broadcast(rbB[:], rs_row[:])
                        rb = bp.tile([P, TT], f32, tag="rb")
                        nc.vector.reciprocal_approx_fast(rb[:], rbB[:])
                        if DBG and blk == 0 and h == 0:
                            nc.sync.dma_start(dbg["dbg_rb"][:], rb[:])
                        for sc in range(2):
                            sl = exp1[sc][:, h * TT:(h + 1) * TT]
                            nc.vector.tensor_mul(sl, sl, rb[:])
                    if DBG and blk == 0:
                        for sc in range(2):
                            nc.sync.dma_start(dbg["dbg_en"][sc * P:(sc + 1) * P, :],
                                              exp1[sc].bitcast(f32)[:])

                    # out_v^T accumulate all heads + bov, then transpose out
                    ovT = [bp1.tile([P, TT], f32, tag=f"ovT{mv}", name=f"ovT{mv}")
                           for mv in range(2)]
                    for mv in range(2):
                        po = ps1.tile([P, TT], f32, tag="mm")
                        for h in range(H):
                            for sc in range(2):
                                nc.tensor.matmul(
                                    po[:],
                                    Pm[sc][:, h * VD + mv * P:h * VD + (mv + 1) * P],
                                    exp1[sc][:, h * TT:(h + 1) * TT],
                                    start=(h == 0 and sc == 0), stop=False)
                        nc.tensor.matmul(po[:], bov_r[:, mv * P:(mv + 1) * P],
                                         onest[:], start=False, stop=True)
                        nc.any.tensor_copy(ovT[mv][:], po[:])
                    if DBG and blk == 0:
                        for mv in range(2):
                            nc.sync.dma_start(dbg["dbg_ovt"][mv * P:(mv + 1) * P, :],
                                              ovT[mv][:])
                    for tt in range(NTT):
                        ovf = bp.tile([P, VD], f32, tag="ovf")
                        for mv in range(2):
                            pt = ps1.tile([P, P], f32, tag="mm")
                            nc.tensor.transpose(
                                pt[:], ovT[mv][:, tt * P:(tt + 1) * P], identf[:])
                            nc.any.tensor_copy(ovf[:, mv * P:(mv + 1) * P], pt[:])
                        nc.sync.dma_start(
                            ov_ap[t0 + tt * P:t0 + (tt + 1) * P, :], ovf[:])

            # ---------- collective (emitted inside last block) ----------
            if DBG:
                for d in range(2):
                    nc.sync.dma_start(dbg["dbg_aacc"][d * P:(d + 1) * P, :],
                                      Aacc[d][:])
                for sc in range(2):
                    nc.sync.dma_start(dbg["dbg_cs"][sc * P:(sc + 1) * P, :],
                                      cs2[sc][:])
            mid.__exit__(None, None, None)

            # ---------- end phase: out_l ----------
            with tc.tile_pool(name="endp", bufs=2) as ep, \
                 tc.tile_pool(name="endp1", bufs=1) as ep1, \
                 tc.tile_pool(name="pse", bufs=2, space="PSUM") as pse, \
                 tc.tile_pool(name="pser", bufs=2, space="PSUM") as pser:
                Ag = [ep1.tile([P, H * TL], f32, tag=f"ag{d}", name=f"ag{d}")
                      for d in range(2)]
                for d in range(2):
                    for qq in range(4):
                        eng = (nc.gpsimd, nc.scalar, nc.gpsimd, nc.sync)[qq]
                        sl = slice(qq * 2 * TL, (qq + 1) * 2 * TL)
                        eng.dma_start(Ag[d][:, sl], cc_out[d, :, sl])
                csg = [ep1.tile([P, H], f32, tag=f"csg{s}", name=f"csg{s}")
                       for s in range(2)]
                for sc in range(2):
                    nc.sync.dma_start(csg[sc][:], cc_out[sc, :, H * TL:ccw])

                csT = [ep1.tile([H, P], f32, tag=f"csT{s}", name=f"csT{s}")
                       for s in range(2)]
                for sc in range(2):
                    pt = pse.tile([H, P], f32, tag="mm")
                    nc.tensor.transpose(pt[:], csg[sc][:], identf[:])
                    nc.any.tensor_copy(csT[sc][:], pt[:])
                csrow = ep1.tile([H, TL], f32, tag="csrow")
                for sc in range(2):
                    nc.sync.dma_start(csrow[:, sc * P:(sc + 1) * P], csT[sc][:])
                crec = ep1.tile([H, TL], f32, tag="crec")
                nc.vector.reciprocal_approx_fast(crec[:], csrow[:])

                Ab = [ep1.tile([P, H * TL], f32r, tag=f"ab{d}", name=f"ab{d}")
                      for d in range(2)]
                for h in range(H):
                    crowf = ep.tile([1, TL], f32, tag="crowf", name="crowf")
                    nc.sync.dma_start(crowf[:], crec[h:h + 1, :])
                    pb = ep.tile([P, TL], f32, tag="pbb", name="pbb")
                    nc.gpsimd.partition_broadcast(pb[:], crowf[:])
                    for d in range(2):
                        nc.vector.tensor_mul(Ab[d][:, h * TL:(h + 1) * TL],
                                             Ag[d][:, h * TL:(h + 1) * TL], pb[:])

                Wvv_r = []
                for d in range(2):
                    st = ep.tile([P, E], f32, tag="wvvst")
                    nc.sync.dma_start(st[:], Wvv_ap[d * P:(d + 1) * P, :])
                    rt = ep1.tile([P, E], f32r, tag=f"wvvr{d}", name=f"wvvr{d}")
                    nc.vector.tensor_copy(rt[:], st[:])
                    Wvv_r.append(rt)
                vvT = [ep1.tile([P, TL], f32r, tag=f"vvT{m}", name=f"vvT{m}")
                       for m in range(EC)]
                for h in range(H):
                    for me in range(2):
                        pv = pse.tile([P, TL], f32, tag="mm")
                        for d in range(2):
                            nc.tensor.matmul(
                                pv[:],
                                Wvv_r[d][:, h * D + me * P:h * D + (me + 1) * P],
                                Ab[d][:, h * TL:(h + 1) * TL],
                                start=(d == 0), stop=(d == 1))
                        nc.any.tensor_copy(vvT[2 * h + me][:], pv[:])

                NB = 2
                NW = LD // NB  # 384
                pc = [pser.tile([1, NW], f32, tag="rs", name="pcrow")
                      for _ in range(NB)]
                po = [pse.tile([P, NW], f32, tag="po", name="po", bufs=4)
                      for _ in range(4)]
                for c in range(EC):
                    st = ep.tile([P, LD], f32, tag="wolst", name="wolst", bufs=3)
                    nc.sync.dma_start(st[:], Wol_ap[c * P:(c + 1) * P, :])
                    rt = ep.tile([P, LD], f32r, tag="wolr", name="wolr", bufs=3)
                    nc.vector.tensor_copy(rt[:], st[:])
                    for nb in range(NB):
                        nc.tensor.matmul(pc[nb][:], bvv_r[:, c:c + 1],
                                         rt[:, nb * NW:(nb + 1) * NW],
                                         start=(c == 0), stop=(c == EC - 1))
                        for sh in range(2):
                            nc.tensor.matmul(
                                po[sh * NB + nb][:], vvT[c][:, sh * P:(sh + 1) * P],
                                rt[:, nb * NW:(nb + 1) * NW],
                                start=(c == 0), stop=False)
                c_f = ep1.tile([1, LD], f32, tag="cf")
                for nb in range(NB):
                    nc.scalar.copy(c_f[:, nb * NW:(nb + 1) * NW], pc[nb][:])
                nc.vector.tensor_add(c_f[:], c_f[:], bol_f[:])
                c_r = ep1.tile([1, LD], f32r, tag="cr")
                nc.vector.tensor_copy(c_r[:], c_f[:])

                for sh in range(2):
                    olf = ep1.tile([P, LD], f32, tag=f"olf{sh}", name=f"olf{sh}")
                    for nb in range(NB):
                        nc.tensor.matmul(po[sh * NB + nb][:], ones1[:],
                                         c_r[:, nb * NW:(nb + 1) * NW],
                                         start=False, stop=True)
                        nc.any.tensor_copy(olf[:, nb * NW:(nb + 1) * NW],
                                           po[sh * NB + nb][:])
                    nc.sync.dma_start(ol_ap[sh * P:(sh + 1) * P, :], olf[:])

    nc.compile()
    return nc


def kernel(v, l, attention_mask_l, Wv, bv, Wl, bl, Wvv, bvv, Wvl, bvl,
           Wov, bov, Wol, bol):
    if "nc" not in _CACHED:
        _CACHED["nc"] = build_nc()
    nc = _CACHED["nc"]

    v = np.asarray(v, dtype=np.float32)
    l = np.asarray(l, dtype=np.float32)
    maskf = np.asarray(attention_mask_l).astype(np.float32)
    w = {k: np.ascontiguousarray(np.asarray(x, dtype=np.float32))
         for k, x in (("Wv", Wv), ("Wl", Wl), ("Wvl", Wvl), ("Wov", Wov),
                      ("Wvv", Wvv), ("Wol", Wol))}
    bias = {"bv": bv, "bl": bl, "bvl": bvl, "bvv": bvv}
    bias = {k: np.ascontiguousarray(np.asarray(x, np.float32).reshape(E, 1))
            for k, x in bias.items()}
    bov_m = np.ascontiguousarray(np.asarray(bov, np.float32).reshape(1, VD))
    bol_m = np.ascontiguousarray(np.asarray(bol, np.float32).reshape(1, LD))

    in_maps = []
    for c in range(N_CORES):
        b = c // 4
        s = c % 4
        m = {
            "v": np.ascontiguousarray(v[b, s * TVC:(s + 1) * TVC, :]),
            "l": np.ascontiguousarray(l[b]),
            "mask": np.ascontiguousarray(maskf[b].reshape(TL, 1)),
            "bov": bov_m, "bol": bol_m,
        }
        m.update(w)
        m.update(bias)
        in_maps.append(m)

    _CACHED["in_maps"] = in_maps
    res = run_bass_kernel_spmd(nc, in_maps, core_ids=list(range(N_CORES)))
    out_v = np.empty((B, TV, VD), np.float32)
    for c in range(N_CORES):
        b, s = c // 4, c % 4
        out_v[b, s * TVC:(s + 1) * TVC, :] = res.results[c]["out_v"]
    out_l = np.stack([res.results[0]["out_l"], res.results[4]["out_l"]])
    return out_v, out_l
